# revision 3
# baseline (speedup 1.0000x reference)
"""nn_ContrastiveLoss Trainium2 kernel (8 NeuronCores, data-parallel over batch).

Contract: kernel(embeddings=[64,1024,128] f32, labels=[64,1024] int64) -> f32 scalar.

Sharding: batch dim B=64 split as 8 samples per core. Host packs each sample's
rows by label (positives first, then negatives, each zero-padded to a 128-row
multiple) AND permutes rows so the device DMA is fully contiguous per
partition (device tile [p, t] = packed row t*128+p lives at host row p*tt+t).

Device pipeline per sample (engine-balanced):
  - DMA e_nat [128, tt, 128] f32 (5KB contiguous per partition)
  - ACT Square -> esq bf16; DVE reduce -> nsq; ACT sqrt(+eps^2); DVE recip
  - GPSIMD tensor_mul: e_nrm_bf16 = e_nat * rinv (free-broadcast) — fuses
    normalize + bf16 cast off the ACT/DVE critical path
  - PE: 10 identity-transposes e_nrm^T -> PSUM fp32 (pos half, neg half)
  - ACT copy (pos) / DVE copy (neg) -> et_p, et_n bf16 SBUF
  - PE sim matmuls: et_p_tile^T @ et_n -> PSUM fp32 [128, padn]
  - hinge fused with reduce via accum_out: even mt slots on DVE
    (max(sim,0.15) sum, constant offset removed host-side), odd on ACT
    (relu(sim-0.15) sum)
  - tail: ones^T @ slots matmul = partition reduction; DMA [1, bpc*tp] raw
    slot sums. Host: subtract DVE offsets, divide by per-sample max(nneg,1),
    apply validity, divide by total count (the all-reduce + division of the
    sharding hint, plus count bookkeeping, all exact host arithmetic).
"""

import sys

if "/opt/trn_rl_repo" not in sys.path:
    sys.path.insert(0, "/opt/trn_rl_repo")

from contextlib import ExitStack

import numpy as np

import concourse.bass as bass
import concourse.bacc as bacc
import concourse.mybir as mybir
import concourse.tile as tile
from concourse import bass_utils

F32 = mybir.dt.float32
BF16 = mybir.dt.bfloat16
AF = mybir.ActivationFunctionType
ALU = mybir.AluOpType

P = 128      # SBUF partitions
D = 128      # embedding dim
N = 1024     # rows per sample
B = 64       # full batch
NCORES = 8
BPC = B // NCORES
THRESH = 0.5 - 0.35   # margin threshold 0.15
EPS = 1e-6


def _kernel_body(ctx, tc, emb_ap, out_ap, bpc, padp, padn):
    nc = tc.nc
    tp, tn = padp // P, padn // P
    tt = tp + tn

    const_pool = ctx.enter_context(tc.tile_pool(name="const", bufs=1))
    epool = ctx.enter_context(tc.tile_pool(name="epool", bufs=3))
    etpool = ctx.enter_context(tc.tile_pool(name="etpool", bufs=2))
    small = ctx.enter_context(tc.tile_pool(name="small", bufs=2))
    acc_pool = ctx.enter_context(tc.tile_pool(name="acc", bufs=1))
    tr_psum = ctx.enter_context(tc.tile_pool(name="trps", bufs=2, space="PSUM"))
    sim_psum = ctx.enter_context(tc.tile_pool(name="simps", bufs=2, space="PSUM"))

    neg_thr = const_pool.tile([P, 1], F32)
    nc.gpsimd.memset(neg_thr[:], -THRESH)
    eps2 = const_pool.tile([P, 1], F32)
    nc.gpsimd.memset(eps2[:], EPS * EPS)
    ones_col = const_pool.tile([P, 1], F32)
    nc.gpsimd.memset(ones_col[:], 1.0)
    # bf16 identity for PE transposes
    ident = const_pool.tile([P, D], BF16)
    nc.gpsimd.affine_select(
        ident[:], ones_col[:].broadcast_to([P, D]),
        pattern=[[-1, D]], compare_op=ALU.is_equal, fill=0.0,
        base=0, channel_multiplier=1,
    )

    # Dummy activations to pull both ACT table loads into the initial DMA wait.
    warm = const_pool.tile([P, 1], F32)
    nc.scalar.activation(warm[:], eps2[:], AF.Square)
    nc.scalar.activation(warm[:], eps2[:], AF.Sqrt, bias=eps2[:])

    slots_all = acc_pool.tile([P, bpc, tp], F32)

    for b in range(bpc):
        e_nat = epool.tile([P, tt, D], F32, tag="e_nat")
        nc.sync.dma_start(e_nat[:], emb_ap[b])

        esq = epool.tile([P, tt, D], BF16, tag="esq")
        nc.scalar.activation(esq[:], e_nat[:], AF.Square)
        nsq = small.tile([P, tt], F32, tag="nsq")
        nc.vector.tensor_reduce(nsq[:], esq[:], axis=mybir.AxisListType.X,
                                op=ALU.add)
        # r = sqrt(nsq + eps^2) folds in the max(r, eps) clamp (pad rows)
        r_ = small.tile([P, tt], F32, tag="r_")
        nc.scalar.activation(r_[:], nsq[:], AF.Sqrt, bias=eps2[:])
        rinv = small.tile([P, tt], F32, tag="rinv")
        nc.vector.reciprocal(rinv[:], r_[:])

        # normalized bf16 rows in one gpsimd op (frees ACT/DVE)
        e_nrm = epool.tile([P, tt, D], BF16, tag="e_nrm")
        nc.gpsimd.tensor_mul(e_nrm[:], e_nat[:],
                             rinv[:].unsqueeze(2).broadcast_to([P, tt, D]))

        # plain PE transposes vs constant identity -> fp32 PSUM
        ps_p = tr_psum.tile([P, padp], F32, tag="trps")
        ps_n = tr_psum.tile([P, padn], F32, tag="trps")
        for t in range(tp):
            nc.tensor.matmul(ps_p[:, bass.ts(t, P)], lhsT=e_nrm[:, t, :],
                             rhs=ident[:], start=True, stop=True)
        for t in range(tn):
            nc.tensor.matmul(ps_n[:, bass.ts(t, P)], lhsT=e_nrm[:, tp + t, :],
                             rhs=ident[:], start=True, stop=True)
        et_p = etpool.tile([P, padp], BF16, tag="et_p")
        nc.scalar.copy(et_p[:], ps_p[:])
        et_n = etpool.tile([P, padn], BF16, tag="et_n")
        nc.vector.tensor_copy(et_n[:], ps_n[:])

        # sim matmuls + fused hinge reduction (sim fully normalized)
        for mt in range(tp):
            sim_ps = sim_psum.tile([P, padn], F32, tag="simps")
            for j0 in range(0, padn, 512):
                jw = min(512, padn - j0)
                nc.tensor.matmul(sim_ps[:, j0:j0 + jw],
                                 lhsT=et_p[:, bass.ts(mt, P)],
                                 rhs=et_n[:, j0:j0 + jw],
                                 start=True, stop=True)
            slot = slots_all[:, b, mt:mt + 1]
            if mt % 2 == 1:
                nc.scalar.activation(sim_ps[:], sim_ps[:], AF.Relu,
                                     bias=neg_thr[:], accum_out=slot)
            else:
                nc.vector.tensor_scalar(sim_ps[:], sim_ps[:], THRESH, None,
                                        ALU.max, ALU.add, accum_out=slot)

    # partition-reduce all slots with one tiny fp32 matmul: ones^T @ slots
    red_ps = sim_psum.tile([1, bpc * tp], F32, tag="simps")
    nc.tensor.matmul(red_ps[:], lhsT=ones_col[:],
                     rhs=slots_all[:].rearrange("p b t -> p (b t)"),
                     start=True, stop=True)
    out_sb = small.tile([1, bpc * tp], F32, tag="out_sb")
    nc.scalar.copy(out_sb[:], red_ps[:])
    nc.sync.dma_start(out_ap[:], out_sb[:])


_NC_CACHE = {}


def _build(padp, padn):
    key = (BPC, NCORES, padp, padn)
    if key in _NC_CACHE:
        return _NC_CACHE[key]
    tp = padp // P
    nc = bacc.Bacc("TRN2", target_bir_lowering=False, debug=False,
                   num_devices=NCORES)
    tt = (padp + padn) // P
    emb = nc.dram_tensor("emb", [BPC, P, tt, D], F32, kind="ExternalInput")
    out = nc.dram_tensor("out", [1, BPC * tp], F32, kind="ExternalOutput")
    with tile.TileContext(nc) as tc:
        with ExitStack() as ctx:
            _kernel_body(ctx, tc, emb.ap(), out.ap(), BPC, padp, padn)
    nc.compile()
    _NC_CACHE[key] = nc
    return nc


def _pack(emb, labels):
    """Per-sample label packing: pos rows, zero pad, neg rows, zero pad.

    Rows are additionally permuted so that the device-side DMA of tile
    [p, t] (= packed row t*128+p) reads contiguously: host row p*tt + t.
    """
    npos = (labels == 1).sum(axis=1)
    nneg = (labels == 0).sum(axis=1)
    padp = max(P, int(-(-npos.max() // P)) * P)
    padn = max(P, int(-(-nneg.max() // P)) * P)
    tt = (padp + padn) // P
    packed = np.zeros((B, padp + padn, D), np.float32)
    for b in range(B):
        pos_idx = np.nonzero(labels[b] == 1)[0]
        neg_idx = np.nonzero(labels[b] == 0)[0]
        packed[b, :len(pos_idx)] = emb[b, pos_idx]
        packed[b, padp:padp + len(neg_idx)] = emb[b, neg_idx]
    # [B, tt*P, D] -> [B, tt, P, D] -> [B, P, tt, D]: row t*128+p -> [p, t]
    perm = np.ascontiguousarray(
        packed.reshape(B, tt, P, D).transpose(0, 2, 1, 3))
    return perm, padp, padn, npos, nneg


def kernel(embeddings: np.ndarray, labels: np.ndarray,
           _want_results=False, _trace=False) -> np.ndarray:
    emb = np.ascontiguousarray(embeddings, dtype=np.float32)
    lab = np.asarray(labels)
    assert emb.shape == (B, N, D) and lab.shape == (B, N)

    perm, padp, padn, npos, nneg = _pack(emb, lab)
    tp = padp // P
    nc = _build(padp, padn)
    in_maps = [{"emb": perm[c * BPC:(c + 1) * BPC]} for c in range(NCORES)]
    res = bass_utils.run_bass_kernel_spmd(nc, in_maps,
                                          core_ids=list(range(NCORES)),
                                          trace=_trace)

    # host-side: remove DVE max-trick offsets, per-sample division, all-reduce
    n_dve = len(range(0, tp, 2))
    dve_off = float(n_dve) * float(P) * float(padn) * THRESH
    loss_sum = 0.0
    for c in range(NCORES):
        slots = np.asarray(res.results[c]["out"], np.float64).reshape(BPC, tp)
        s_raw = slots.sum(axis=1) - dve_off
        for i in range(BPC):
            b = c * BPC + i
            if npos[b] > 0 and nneg[b] > 0:
                loss_sum += s_raw[i] / max(float(nneg[b]), 1.0)
    valid = (npos > 0) & (nneg > 0)
    count = float((npos * valid).sum())
    ans = np.float32(loss_sum / max(count, 1.0))
    if _want_results:
        return ans, res
    return ans


# revision 5
# speedup vs baseline: 1.2294x; 1.2294x over previous
"""nn_ContrastiveLoss Trainium2 kernel (8 NeuronCores, data-parallel over batch).

Contract: kernel(embeddings=[64,1024,128] f32, labels=[64,1024] int64) -> f32 scalar.

Sharding: batch dim B=64 split as 8 samples per core. Host packs each sample's
rows by label (positives first, then negatives, each zero-padded to a 128-row
multiple) AND permutes rows so the device-side DMA is fully contiguous per
partition (device tile [p, t] = packed row t*128+p lives at host row p*tt+t).

Device pipeline, software-pipelined 3 deep (issue order interleaves samples so
per-engine FIFOs don't serialize the chain):
  phase A(b):   DMA e_nat [128, tt, 128] f32 (5KB contiguous per partition)
  phase B(b-1): ACT Square -> esq bf16; DVE reduce -> nsq; ACT sqrt(+eps^2);
                DVE recip; GPSIMD tensor_mul e_nrm = e_nat * rinv (bf16);
                PE transpose-mode (bf16 PSUM, 1 bank per half);
                DVE 2x copies -> et_p/et_n bf16 SBUF
  phase C(b-2): PE sim matmuls into grouped PSUM tiles [128, 2, 640];
                hinge fused with accum_out: ACT relu(sim-0.15) on groups
                {0,1} and {4}, DVE max(sim,0.15)-sum on group {2,3}
                (constant offset removed host-side)
  tail: ones^T @ slots matmul = partition reduction; DMA [1, bpc*tp] raw slot
  sums. Host: subtract DVE offsets, per-sample division by max(nneg,1),
  validity, and the final count division (exact host arithmetic; counts come
  from labels).
"""

import sys

if "/opt/trn_rl_repo" not in sys.path:
    sys.path.insert(0, "/opt/trn_rl_repo")

from contextlib import ExitStack

import numpy as np

import concourse.bass as bass
import concourse.bacc as bacc
import concourse.mybir as mybir
import concourse.tile as tile
from concourse import bass_utils

F32 = mybir.dt.float32
BF16 = mybir.dt.bfloat16
AF = mybir.ActivationFunctionType
ALU = mybir.AluOpType

P = 128      # SBUF partitions
D = 128      # embedding dim
N = 1024     # rows per sample
B = 64       # full batch
NCORES = 8
BPC = B // NCORES
THRESH = 0.5 - 0.35   # margin threshold 0.15
EPS = 1e-6


def _kernel_body(ctx, tc, emb_ap, out_ap, bpc, padp, padn):
    nc = tc.nc
    tp, tn = padp // P, padn // P
    tt = tp + tn

    const_pool = ctx.enter_context(tc.tile_pool(name="const", bufs=1))
    epool = ctx.enter_context(tc.tile_pool(name="epool", bufs=3))
    etpool = ctx.enter_context(tc.tile_pool(name="etpool", bufs=3))
    small = ctx.enter_context(tc.tile_pool(name="small", bufs=3))
    acc_pool = ctx.enter_context(tc.tile_pool(name="acc", bufs=1))
    tr_psum = ctx.enter_context(tc.tile_pool(name="trps", bufs=2, space="PSUM"))
    sim_psum = ctx.enter_context(tc.tile_pool(name="simps", bufs=2, space="PSUM"))

    neg_thr = const_pool.tile([P, 1], F32)
    nc.gpsimd.memset(neg_thr[:], -THRESH)
    eps2 = const_pool.tile([P, 1], F32)
    nc.gpsimd.memset(eps2[:], EPS * EPS)
    ones_col = const_pool.tile([P, 1], F32)
    nc.gpsimd.memset(ones_col[:], 1.0)
    # bf16 identity for PE transpose mode
    ident = const_pool.tile([P, D], BF16)
    nc.gpsimd.affine_select(
        ident[:], ones_col[:].broadcast_to([P, D]),
        pattern=[[-1, D]], compare_op=ALU.is_equal, fill=0.0,
        base=0, channel_multiplier=1,
    )

    # Dummy activations to pull both ACT table loads into the initial DMA wait.
    warm = const_pool.tile([P, 1], F32)
    nc.scalar.activation(warm[:], eps2[:], AF.Square)
    nc.scalar.activation(warm[:], eps2[:], AF.Sqrt, bias=eps2[:])

    slots_all = acc_pool.tile([P, bpc, tp], F32)

    # hinge slot groups: (engine, [mt...]) — ACT relu groups, DVE max group
    groups = []
    mt = 0
    while mt < tp:
        w = min(2, tp - mt)
        eng = "DVE" if mt == 2 else "ACT"
        groups.append((eng, list(range(mt, mt + w))))
        mt += w

    e_nats, esqs, nsqs, rs, rinvs, e_nrms = {}, {}, {}, {}, {}, {}
    ets = {}

    def phase_dma(b):
        e_nat = epool.tile([P, tt, D], F32, tag="e_nat", name=f"e_nat{b}")
        nc.sync.dma_start(e_nat[:], emb_ap[b])
        e_nats[b] = e_nat

    def phase_norm(b):
        e_nat = e_nats[b]
        esq = epool.tile([P, tt, D], BF16, tag="esq", name=f"esq{b}")
        nc.scalar.activation(esq[:], e_nat[:], AF.Square)
        nsq = small.tile([P, tt], F32, tag="nsq", name=f"nsq{b}")
        nc.vector.tensor_reduce(nsq[:], esq[:], axis=mybir.AxisListType.X,
                                op=ALU.add)
        # r = sqrt(nsq + eps^2) folds in the max(r, eps) clamp (pad rows)
        r_ = small.tile([P, tt], F32, tag="r_", name=f"r{b}")
        nc.scalar.activation(r_[:], nsq[:], AF.Sqrt, bias=eps2[:])
        rinv = small.tile([P, tt], F32, tag="rinv", name=f"rinv{b}")
        nc.vector.reciprocal(rinv[:], r_[:])

        # normalized bf16 rows in one gpsimd op (frees ACT/DVE)
        e_nrm = epool.tile([P, tt, D], BF16, tag="e_nrm", name=f"e_nrm{b}")
        nc.gpsimd.tensor_mul(e_nrm[:], e_nat[:],
                             rinv[:].unsqueeze(2).broadcast_to([P, tt, D]))

        # PE transpose mode -> bf16 PSUM (1 bank per half)
        ps_p = tr_psum.tile([P, padp], BF16, tag="trps", name=f"psp{b}")
        ps_n = tr_psum.tile([P, padn], BF16, tag="trps", name=f"psn{b}")
        for t in range(tp):
            nc.tensor.transpose(ps_p[:, bass.ts(t, P)], e_nrm[:, t, :],
                                ident[:])
        for t in range(tn):
            nc.tensor.transpose(ps_n[:, bass.ts(t, P)], e_nrm[:, tp + t, :],
                                ident[:])
        et_p = etpool.tile([P, padp], BF16, tag="et_p", name=f"etp{b}")
        nc.vector.tensor_copy(et_p[:], ps_p[:])
        et_n = etpool.tile([P, padn], BF16, tag="et_n", name=f"etn{b}")
        nc.vector.tensor_copy(et_n[:], ps_n[:])
        ets[b] = (et_p, et_n)

    def phase_sim(b):
        et_p, et_n = ets.pop(b)
        for eng, mts in groups:
            gw = len(mts)
            sim_ps = sim_psum.tile([P, 2, padn], F32, tag="simps",
                                   name=f"sim{b}_{mts[0]}")
            for gi, mt in enumerate(mts):
                # chunk at absolute 512-fp32 PSUM bank edges within the tile
                j0 = 0
                while j0 < padn:
                    jw = min(512 - ((gi * padn + j0) % 512), padn - j0)
                    nc.tensor.matmul(sim_ps[:, gi, j0:j0 + jw],
                                     lhsT=et_p[:, bass.ts(mt, P)],
                                     rhs=et_n[:, j0:j0 + jw],
                                     start=True, stop=True)
                    j0 += jw
            view = sim_ps[:, 0:gw, :]
            slot = slots_all[:, b, mts[0]:mts[0] + 1]
            if eng == "ACT":
                nc.scalar.activation(view, view, AF.Relu,
                                     bias=neg_thr[:], accum_out=slot)
            else:
                nc.vector.tensor_scalar(view, view, THRESH, None,
                                        ALU.max, ALU.add, accum_out=slot)

    # software-pipelined issue: DMA(b) | norm(b-1) | sim+hinge(b-2)
    phase_dma(0)
    for s in range(1, bpc + 2):
        if s < bpc:
            phase_dma(s)
        if s - 1 < bpc:
            phase_norm(s - 1)
        if s >= 2:
            phase_sim(s - 2)

    # partition-reduce all slots with one tiny fp32 matmul: ones^T @ slots
    red_ps = sim_psum.tile([1, bpc * tp], F32, tag="simps")
    nc.tensor.matmul(red_ps[:], lhsT=ones_col[:],
                     rhs=slots_all[:].rearrange("p b t -> p (b t)"),
                     start=True, stop=True)
    out_sb = small.tile([1, bpc * tp], F32, tag="out_sb")
    nc.scalar.copy(out_sb[:], red_ps[:])
    nc.sync.dma_start(out_ap[:], out_sb[:])


_NC_CACHE = {}


def _build(padp, padn):
    key = (BPC, NCORES, padp, padn)
    if key in _NC_CACHE:
        return _NC_CACHE[key]
    tp = padp // P
    nc = bacc.Bacc("TRN2", target_bir_lowering=False, debug=False,
                   num_devices=NCORES)
    tt = (padp + padn) // P
    emb = nc.dram_tensor("emb", [BPC, P, tt, D], F32, kind="ExternalInput")
    out = nc.dram_tensor("out", [1, BPC * tp], F32, kind="ExternalOutput")
    with tile.TileContext(nc) as tc:
        with ExitStack() as ctx:
            _kernel_body(ctx, tc, emb.ap(), out.ap(), BPC, padp, padn)
    nc.compile()
    _NC_CACHE[key] = nc
    return nc


def _pack(emb, labels):
    """Per-sample label packing: pos rows, zero pad, neg rows, zero pad.

    Rows are additionally permuted so that the device-side DMA of tile
    [p, t] (= packed row t*128+p) reads contiguously: host row p*tt + t.
    """
    npos = (labels == 1).sum(axis=1)
    nneg = (labels == 0).sum(axis=1)
    padp = max(P, int(-(-npos.max() // P)) * P)
    padn = max(P, int(-(-nneg.max() // P)) * P)
    tt = (padp + padn) // P
    packed = np.zeros((B, padp + padn, D), np.float32)
    for b in range(B):
        pos_idx = np.nonzero(labels[b] == 1)[0]
        neg_idx = np.nonzero(labels[b] == 0)[0]
        packed[b, :len(pos_idx)] = emb[b, pos_idx]
        packed[b, padp:padp + len(neg_idx)] = emb[b, neg_idx]
    # [B, tt*P, D] -> [B, tt, P, D] -> [B, P, tt, D]: row t*128+p -> [p, t]
    perm = np.ascontiguousarray(
        packed.reshape(B, tt, P, D).transpose(0, 2, 1, 3))
    return perm, padp, padn, npos, nneg


def _dve_slot_count(tp):
    # mirrors the group assignment in _kernel_body
    n = 0
    mt = 0
    while mt < tp:
        w = min(2, tp - mt)
        if mt == 2:
            n += w
        mt += w
    return n


def kernel(embeddings: np.ndarray, labels: np.ndarray,
           _want_results=False, _trace=False) -> np.ndarray:
    emb = np.ascontiguousarray(embeddings, dtype=np.float32)
    lab = np.asarray(labels)
    assert emb.shape == (B, N, D) and lab.shape == (B, N)

    perm, padp, padn, npos, nneg = _pack(emb, lab)
    tp = padp // P
    nc = _build(padp, padn)
    in_maps = [{"emb": perm[c * BPC:(c + 1) * BPC]} for c in range(NCORES)]
    res = bass_utils.run_bass_kernel_spmd(nc, in_maps,
                                          core_ids=list(range(NCORES)),
                                          trace=_trace)

    # host-side: remove DVE max-trick offsets, per-sample division, all-reduce
    dve_off = float(_dve_slot_count(tp)) * float(P) * float(padn) * THRESH
    loss_sum = 0.0
    for c in range(NCORES):
        slots = np.asarray(res.results[c]["out"], np.float64).reshape(BPC, tp)
        s_raw = slots.sum(axis=1) - dve_off
        for i in range(BPC):
            b = c * BPC + i
            if npos[b] > 0 and nneg[b] > 0:
                loss_sum += s_raw[i] / max(float(nneg[b]), 1.0)
    valid = (npos > 0) & (nneg > 0)
    count = float((npos * valid).sum())
    ans = np.float32(loss_sum / max(count, 1.0))
    if _want_results:
        return ans, res
    return ans
